# revision 12
# baseline (speedup 1.0000x reference)
"""Trainium2 Bass kernel for the BINN convnet problem.

Computation (per row b of inp, all column indices mod D=128):
    x[b, j]  = (c1[j] * a[b, j+1] - c2[j] * a[b, j-2]) * a[b, j-1]
    out      = x + a @ W_lin.T + b_lin
with c1[j] = w[j,0]*w[j,2], c2[j] = w[j,1]*w[j,2], except j==1 where the
outer factor is w[1,0] instead of w[1,2].

Strategy: pure data parallel across 8 NeuronCores (batch split).  On each
core, per 512-row compute tile (1024-row DMA tiles):
  1. DMA the natural-layout rows into SBUF ([128 partitions x row-blocks]).
  2. PE-transpose to A^T (d on partitions), evacuate PSUM->SBUF on ScalarE
     (rounding to float32r on the way).
  3. The stencil's linear part is a constant banded matrix:
       g[b, j] = c1[j] a[b, j+1] - c2[j] a[b, j-2]  ==  (A @ G^T)[b, j].
     In transposed layout it is one 128x128 constant float32r matmul.  We
     use the row-rotated G~ (G~[i,:] = G[i+1,:]) so the remaining
     a[b, j-1] factor is a *partition-aligned* elementwise multiply:
       gtld[i, b] = g[b, i+1] ;  x~[i, b] = A^T[i, b] * gtld[i, b]   (DVE)
  4. mm~ = W~ @ A^T on PE (float32r, N=512 -> full rate); s~ = x~ + mm~ (DVE).
  5. PE transposes s~ back to natural layout THROUGH A ROTATED PERMUTATION
     (transpose-mode rhs = rotation matrix, out[b, i+1] = s~[i, b]), which
     undoes the G~/W~ rotation for free, accumulating in PSUM on top of a
     K=1 bias matmul (ones^T x b_lin).
  6. Plain ScalarE evacuation, DMA the natural tile back out.
"""

import os
import sys

import numpy as np

if os.path.isdir("/opt/trn_rl_repo") and "/opt/trn_rl_repo" not in sys.path:
    sys.path.insert(0, "/opt/trn_rl_repo")

import concourse.mybir as mybir
import concourse.tile as tile
from concourse import bacc
from concourse.bass_utils import run_bass_kernel_spmd

D = 128          # feature dim
N_CORES = 8
SUB = 512        # rows per compute subtile (one PSUM bank wide)
DMA_ROWS = 1024  # rows per DMA tile
F32 = mybir.dt.float32
F32R = mybir.dt.float32r


def build_program(nrows: int):
    """Build the single-core Bass program (SPMD across cores)."""
    assert nrows % DMA_ROWS == 0
    ndma = nrows // DMA_ROWS
    nsub = DMA_ROWS // SUB  # compute subtiles per DMA tile (2)
    QB = SUB // D           # 128-row blocks per compute subtile (4)

    nc = bacc.Bacc("TRN2", debug=False, target_bir_lowering=False)

    inp = nc.declare_dram_parameter("inp", [nrows, D], F32, isOutput=False)
    gt = nc.declare_dram_parameter("gt", [D, D], F32, isOutput=False)
    wt = nc.declare_dram_parameter("wt", [D, D], F32, isOutput=False)
    bvec = nc.declare_dram_parameter("bvec", [1, SUB], F32, isOutput=False)
    ones = nc.declare_dram_parameter("ones", [1, D], F32, isOutput=False)
    ident = nc.declare_dram_parameter("ident", [D, D], F32, isOutput=False)
    rot = nc.declare_dram_parameter("rot", [D, D], F32, isOutput=False)
    out = nc.declare_dram_parameter("out", [nrows, D], F32, isOutput=True)

    with tile.TileContext(nc) as tc:
        with (
            tc.tile_pool(name="const", bufs=1) as const_pool,
            tc.tile_pool(name="a_sb", bufs=3) as a_pool,
            tc.tile_pool(name="at_sb", bufs=3) as at_pool,
            tc.tile_pool(name="xt_sb", bufs=3) as xt_pool,
            tc.tile_pool(name="st_sb", bufs=3) as st_pool,
            tc.tile_pool(name="o_sb", bufs=3) as o_pool,
            tc.tile_pool(name="at_ps", bufs=2, space="PSUM") as atps_pool,
            tc.tile_pool(name="g_ps", bufs=2, space="PSUM") as gps_pool,
            tc.tile_pool(name="m_ps", bufs=2, space="PSUM") as mps_pool,
            tc.tile_pool(name="o_ps", bufs=2, space="PSUM") as ops_pool,
        ):
            # --- constants, loaded once ---
            gt_sb = const_pool.tile([D, D], F32)
            wt_sb = const_pool.tile([D, D], F32)
            b_sb = const_pool.tile([1, SUB], F32)
            ones_sb = const_pool.tile([1, D], F32)
            id_sb = const_pool.tile([D, D], F32)
            rot_sb = const_pool.tile([D, D], F32)
            nc.sync.dma_start(out=gt_sb[:], in_=gt[:, :])
            nc.sync.dma_start(out=wt_sb[:], in_=wt[:, :])
            nc.sync.dma_start(out=b_sb[:], in_=bvec[:, :])
            nc.sync.dma_start(out=ones_sb[:], in_=ones[:, :])
            nc.sync.dma_start(out=id_sb[:], in_=ident[:, :])
            nc.sync.dma_start(out=rot_sb[:], in_=rot[:, :])

            # fp32r matmul operands must be produced by an fp32r-rounding
            # instruction (walrus checkMatmultFP32r) — round the constants once.
            gt_rt = const_pool.tile([D, D], F32R)
            wt_rt = const_pool.tile([D, D], F32R)
            b_rt = const_pool.tile([1, SUB], F32R)
            ones_rt = const_pool.tile([1, D], F32R)
            nc.vector.tensor_copy(out=gt_rt[:], in_=gt_sb[:])
            nc.vector.tensor_copy(out=wt_rt[:], in_=wt_sb[:])
            nc.vector.tensor_copy(out=b_rt[:], in_=b_sb[:])
            nc.vector.tensor_copy(out=ones_rt[:], in_=ones_sb[:])

            for td in range(ndma):
                r0 = td * DMA_ROWS
                # --- load: SBUF[p, q*D + d] = inp[r0 + q*128 + p, d]
                a_sb = a_pool.tile([D, DMA_ROWS], F32, tag="a")
                src = inp[r0 : r0 + DMA_ROWS, :].rearrange("(q p) d -> p q d", p=D)
                nc.sync.dma_start(
                    out=a_sb[:].rearrange("p (q d) -> p q d", d=D), in_=src
                )
                o_sb = o_pool.tile([D, DMA_ROWS], F32, tag="o")

                for ts in range(nsub):
                    sub = slice(ts * SUB, (ts + 1) * SUB)

                    # --- PE transpose A -> A^T (per 128 block, plain fp32) ---
                    at_ps = atps_pool.tile([D, SUB], F32, tag="atps")
                    for q in range(QB):
                        fq = ts * SUB + q * D
                        nc.tensor.matmul(
                            out=at_ps[:, q * D : (q + 1) * D],
                            lhsT=a_sb[:, fq : fq + D],
                            rhs=id_sb[:],
                            is_transpose=True,
                            start=True,
                            stop=True,
                        )
                    # evacuate A^T to SBUF (ScalarE), rounding to fp32r
                    at_sb = at_pool.tile([D, SUB], F32R, tag="at")
                    nc.scalar.copy(out=at_sb[:], in_=at_ps[:])
                    at_r = at_sb[:]

                    # --- stencil linear part: gtld = G~ @ A^T  (PSUM) ---
                    g_ps = gps_pool.tile([D, SUB], F32, tag="gps")
                    nc.tensor.matmul(
                        out=g_ps[:], lhsT=gt_rt[:], rhs=at_r, start=True, stop=True
                    )

                    # --- x~ = A^T * gtld (DVE; one PSUM operand) ---
                    xt_sb = xt_pool.tile([D, SUB], F32, tag="xt")
                    nc.vector.tensor_mul(
                        out=xt_sb[:], in0=at_sb[:].bitcast(F32), in1=g_ps[:]
                    )

                    # --- mm~ = W~ @ A^T (PSUM) ---
                    m_ps = mps_pool.tile([D, SUB], F32, tag="mps")
                    nc.tensor.matmul(
                        out=m_ps[:], lhsT=wt_rt[:], rhs=at_r, start=True, stop=True
                    )

                    # --- s~ = x~ + mm~ (DVE) ---
                    st_sb = st_pool.tile([D, SUB], F32, tag="st")
                    nc.vector.tensor_add(out=st_sb[:], in0=xt_sb[:], in1=m_ps[:])

                    # --- natural-layout accumulation in PSUM: bias + rot(s~^T) ---
                    o_ps = ops_pool.tile([D, SUB], F32, tag="ops")
                    nc.tensor.matmul(
                        out=o_ps[:], lhsT=ones_rt[:], rhs=b_rt[:],
                        start=True, stop=False,
                    )
                    for q in range(QB):
                        nc.tensor.matmul(
                            out=o_ps[:, q * D : (q + 1) * D],
                            lhsT=st_sb[:, q * D : (q + 1) * D],
                            rhs=rot_sb[:],
                            is_transpose=True,
                            start=False,
                            stop=(q == QB - 1),
                        )

                    # --- evacuate (plain copy; rotation already applied) ---
                    nc.scalar.copy(out=o_sb[:, sub], in_=o_ps[:])

                # --- store ---
                dst = out[r0 : r0 + DMA_ROWS, :].rearrange("(q p) d -> p q d", p=D)
                nc.sync.dma_start(
                    out=dst, in_=o_sb[:].rearrange("p (q d) -> p q d", d=D)
                )

    nc.compile()
    return nc


def make_consts(w: np.ndarray, W_lin: np.ndarray, b_lin: np.ndarray):
    """Host-side constant preparation (all tiny)."""
    w = np.asarray(w, np.float64)
    c1 = w[:, 0] * w[:, 2]
    c2 = w[:, 1] * w[:, 2]
    # column 1 uses w[1,0] as the outer factor (faithful to source)
    c1[1] = w[1, 0] * w[1, 0]
    c2[1] = w[1, 1] * w[1, 0]

    j = np.arange(D)
    G = np.zeros((D, D), np.float64)
    G[j, (j + 1) % D] += c1
    G[j, (j - 2) % D] -= c2

    rotidx = (j + 1) % D  # row rotation: X~[i] = X[i+1]
    Gt = np.ascontiguousarray(G[rotidx, :].T, np.float32)          # [d, i]
    Wt = np.ascontiguousarray(np.asarray(W_lin, np.float64)[rotidx, :].T, np.float32)
    bvec = np.ascontiguousarray(
        np.tile(np.asarray(b_lin, np.float32), SUB // D)[None, :]
    )
    ones = np.ones((1, D), np.float32)
    ident = np.eye(D, dtype=np.float32)
    # transpose-back permutation: out[b, (i+1)%D] = s~[i, b]
    rotm = np.zeros((D, D), np.float32)
    rotm[j, (j + 1) % D] = 1.0
    return {"gt": Gt, "wt": Wt, "bvec": bvec, "ones": ones, "ident": ident,
            "rot": rotm}


_PROGRAM_CACHE: dict[int, object] = {}
TRACE = False      # test-only: capture NTFF profile on the next kernel() call
TRACE_DIR = None   # test-only: where to keep NTFF/perfetto artifacts
LAST_RESULT = None  # test-only: BassKernelResults of the last run


def _get_program(nrows: int):
    if nrows not in _PROGRAM_CACHE:
        _PROGRAM_CACHE[nrows] = build_program(nrows)
    return _PROGRAM_CACHE[nrows]


def kernel(**inputs) -> np.ndarray:
    inp = np.ascontiguousarray(np.asarray(inputs["inp"], np.float32))
    w = np.asarray(inputs["w"], np.float32)
    W_lin = np.asarray(inputs["W_lin"], np.float32)
    b_lin = np.asarray(inputs["b_lin"], np.float32)

    B = inp.shape[0]
    assert inp.shape[1] == D and B % N_CORES == 0
    nrows = B // N_CORES

    consts = make_consts(w, W_lin, b_lin)
    shards = inp.reshape(N_CORES, nrows, D)

    nc = _get_program(nrows)
    in_maps = [{"inp": shards[i], **consts} for i in range(N_CORES)]
    res = run_bass_kernel_spmd(
        nc, in_maps, list(range(N_CORES)), trace=TRACE, tmpdir=TRACE_DIR
    )
    global LAST_RESULT
    LAST_RESULT = res
    return np.concatenate([res.results[i]["out"] for i in range(N_CORES)], axis=0)


if __name__ == "__main__":
    # quick smoke test on random data vs numpy
    rng = np.random.default_rng(0)
    B = N_CORES * DMA_ROWS * 2
    inp = rng.standard_normal((B, D)).astype(np.float32)
    w = rng.random((D, 3)).astype(np.float32)
    W_lin = (rng.standard_normal((D, D)) / np.sqrt(D)).astype(np.float32)
    b_lin = (rng.standard_normal(D) * 0.01).astype(np.float32)
    dt = np.ones(1, np.float32)

    actual = kernel(inp=inp, dt=dt, w=w, W_lin=W_lin, b_lin=b_lin)

    a = inp.astype(np.float64)
    c1 = (w[:, 0] * w[:, 2]).astype(np.float64)
    c2 = (w[:, 1] * w[:, 2]).astype(np.float64)
    c1[1] = w[1, 0] * w[1, 0]
    c2[1] = w[1, 1] * w[1, 0]
    ap1 = np.roll(a, -1, 1)
    am2 = np.roll(a, 2, 1)
    am1 = np.roll(a, 1, 1)
    x = (c1 * ap1 - c2 * am2) * am1
    expected = x + a @ W_lin.astype(np.float64).T + b_lin
    err = np.abs(actual - expected).max() / np.abs(expected).max()
    print("scale-relative absmax err:", err)


# revision 17
# speedup vs baseline: 1.0070x; 1.0070x over previous
"""Trainium2 Bass kernel for the BINN convnet problem.

Computation (per row b of inp, all column indices mod D=128):
    x[b, j]  = (c1[j] * a[b, j+1] - c2[j] * a[b, j-2]) * a[b, j-1]
    out      = x + a @ W_lin.T + b_lin
with c1[j] = w[j,0]*w[j,2], c2[j] = w[j,1]*w[j,2], except j==1 where the
outer factor is w[1,0] instead of w[1,2].

Strategy: pure data parallel across 8 NeuronCores (batch split).  On each
core, per 512-row compute tile (1024-row DMA tiles):
  1. DMA the natural-layout rows into SBUF ([128 partitions x row-blocks]).
  2. PE-transpose to A^T (d on partitions), evacuate PSUM->SBUF on ScalarE
     (rounding to float32r on the way).
  3. The stencil's linear part is a constant banded matrix:
       g[b, j] = c1[j] a[b, j+1] - c2[j] a[b, j-2]  ==  (A @ G^T)[b, j].
     In transposed layout it is one 128x128 constant float32r matmul.  We
     use the row-rotated G~ (G~[i,:] = G[i+1,:]) so the remaining
     a[b, j-1] factor is a *partition-aligned* elementwise multiply:
       gtld[i, b] = g[b, i+1] ;  x~[i, b] = A^T[i, b] * gtld[i, b]   (DVE)
  4. mm~ = W~ @ A^T on PE (float32r, N=512 -> full rate); s~ = x~ + mm~ (DVE).
  5. PE transposes s~ back to natural layout THROUGH A ROTATED PERMUTATION
     (transpose-mode rhs = rotation matrix, out[b, i+1] = s~[i, b]), which
     undoes the G~/W~ rotation for free, accumulating in PSUM on top of a
     K=1 bias matmul (ones^T x b_lin).
  6. Plain ScalarE evacuation, DMA the natural tile back out.
"""

import os
import sys

import numpy as np

if os.path.isdir("/opt/trn_rl_repo") and "/opt/trn_rl_repo" not in sys.path:
    sys.path.insert(0, "/opt/trn_rl_repo")

import concourse.mybir as mybir
import concourse.tile as tile
from concourse import bacc
from concourse.bass_utils import run_bass_kernel_spmd

D = 128          # feature dim
N_CORES = 8
SUB = 512        # rows per compute subtile (one PSUM bank wide)
DMA_ROWS = 1024  # rows per DMA tile
F32 = mybir.dt.float32
F32R = mybir.dt.float32r


def build_program(nrows: int):
    """Build the single-core Bass program (SPMD across cores)."""
    assert nrows % DMA_ROWS == 0
    ndma = nrows // DMA_ROWS
    nsub = DMA_ROWS // SUB  # compute subtiles per DMA tile (2)
    QB = SUB // D           # 128-row blocks per compute subtile (4)

    nc = bacc.Bacc("TRN2", debug=False, target_bir_lowering=False)

    inp = nc.declare_dram_parameter("inp", [nrows, D], F32, isOutput=False)
    gt = nc.declare_dram_parameter("gt", [D, D], F32, isOutput=False)
    wt = nc.declare_dram_parameter("wt", [D, D], F32, isOutput=False)
    bvec = nc.declare_dram_parameter("bvec", [1, SUB], F32, isOutput=False)
    ones = nc.declare_dram_parameter("ones", [1, D], F32, isOutput=False)
    ident = nc.declare_dram_parameter("ident", [D, D], F32, isOutput=False)
    rot = nc.declare_dram_parameter("rot", [D, D], F32, isOutput=False)
    out = nc.declare_dram_parameter("out", [nrows, D], F32, isOutput=True)

    with tile.TileContext(nc) as tc:
        with (
            tc.tile_pool(name="const", bufs=1) as const_pool,
            tc.tile_pool(name="a_sb", bufs=3) as a_pool,
            tc.tile_pool(name="at_sb", bufs=3) as at_pool,
            tc.tile_pool(name="xt_sb", bufs=3) as xt_pool,
            tc.tile_pool(name="st_sb", bufs=3) as st_pool,
            tc.tile_pool(name="o_sb", bufs=3) as o_pool,
            tc.tile_pool(name="at_ps", bufs=2, space="PSUM") as atps_pool,
            tc.tile_pool(name="g_ps", bufs=2, space="PSUM") as gps_pool,
            tc.tile_pool(name="m_ps", bufs=2, space="PSUM") as mps_pool,
            tc.tile_pool(name="o_ps", bufs=2, space="PSUM") as ops_pool,
        ):
            # --- constants, loaded once ---
            gt_sb = const_pool.tile([D, D], F32)
            wt_sb = const_pool.tile([D, D], F32)
            b_sb = const_pool.tile([1, SUB], F32)
            ones_sb = const_pool.tile([1, D], F32)
            id_sb = const_pool.tile([D, D], F32)
            rot_sb = const_pool.tile([D, D], F32)
            nc.sync.dma_start(out=gt_sb[:], in_=gt[:, :])
            nc.sync.dma_start(out=wt_sb[:], in_=wt[:, :])
            nc.sync.dma_start(out=b_sb[:], in_=bvec[:, :])
            nc.sync.dma_start(out=ones_sb[:], in_=ones[:, :])
            nc.sync.dma_start(out=id_sb[:], in_=ident[:, :])
            nc.sync.dma_start(out=rot_sb[:], in_=rot[:, :])

            # fp32r matmul operands must be produced by an fp32r-rounding
            # instruction (walrus checkMatmultFP32r) — round the constants once.
            gt_rt = const_pool.tile([D, D], F32R)
            wt_rt = const_pool.tile([D, D], F32R)
            b_rt = const_pool.tile([1, SUB], F32R)
            ones_rt = const_pool.tile([1, D], F32R)
            rot_rt = const_pool.tile([D, D], F32R)
            nc.vector.tensor_copy(out=gt_rt[:], in_=gt_sb[:])
            nc.vector.tensor_copy(out=wt_rt[:], in_=wt_sb[:])
            nc.vector.tensor_copy(out=b_rt[:], in_=b_sb[:])
            nc.vector.tensor_copy(out=ones_rt[:], in_=ones_sb[:])
            nc.vector.tensor_copy(out=rot_rt[:], in_=rot_sb[:])

            for td in range(ndma):
                r0 = td * DMA_ROWS
                # --- load: SBUF[p, q*D + d] = inp[r0 + q*128 + p, d]
                # (p q) layout: partition p holds DMA_ROWS/128 consecutive DRAM
                # rows -> each partition line is one contiguous DRAM chunk.
                a_sb = a_pool.tile([D, DMA_ROWS], F32, tag="a")
                src = inp[r0 : r0 + DMA_ROWS, :].rearrange("(p q) d -> p q d", p=D)
                nc.sync.dma_start(
                    out=a_sb[:].rearrange("p (q d) -> p q d", d=D), in_=src
                )
                o_sb = o_pool.tile([D, DMA_ROWS], F32, tag="o")

                for ts in range(nsub):
                    sub = slice(ts * SUB, (ts + 1) * SUB)

                    # --- PE transpose A -> A^T (per 128 block, plain fp32) ---
                    at_ps = atps_pool.tile([D, SUB], F32, tag="atps")
                    for q in range(QB):
                        fq = ts * SUB + q * D
                        nc.tensor.matmul(
                            out=at_ps[:, q * D : (q + 1) * D],
                            lhsT=a_sb[:, fq : fq + D],
                            rhs=id_sb[:],
                            is_transpose=True,
                            start=True,
                            stop=True,
                        )
                    # evacuate A^T to SBUF (ScalarE), rounding to fp32r
                    at_sb = at_pool.tile([D, SUB], F32R, tag="at")
                    nc.scalar.copy(out=at_sb[:], in_=at_ps[:])
                    at_r = at_sb[:]

                    # --- stencil linear part: gtld = G~ @ A^T  (PSUM) ---
                    g_ps = gps_pool.tile([D, SUB], F32, tag="gps")
                    nc.tensor.matmul(
                        out=g_ps[:], lhsT=gt_rt[:], rhs=at_r, start=True, stop=True
                    )

                    # --- x~ = A^T * gtld (DVE; one PSUM operand) ---
                    xt_sb = xt_pool.tile([D, SUB], F32, tag="xt")
                    nc.vector.tensor_mul(
                        out=xt_sb[:], in0=at_sb[:].bitcast(F32), in1=g_ps[:]
                    )

                    # --- mm~ = W~ @ A^T (PSUM) ---
                    m_ps = mps_pool.tile([D, SUB], F32, tag="mps")
                    nc.tensor.matmul(
                        out=m_ps[:], lhsT=wt_rt[:], rhs=at_r, start=True, stop=True
                    )

                    # --- s~ = x~ + mm~ (DVE, rounding to fp32r for the
                    # fp32r transpose-back) ---
                    st_sb = st_pool.tile([D, SUB], F32R, tag="st")
                    nc.vector.tensor_add(out=st_sb[:], in0=xt_sb[:], in1=m_ps[:])

                    # --- natural-layout accumulation in PSUM: bias + rot(s~^T) ---
                    o_ps = ops_pool.tile([D, SUB], F32, tag="ops")
                    nc.tensor.matmul(
                        out=o_ps[:], lhsT=ones_rt[:], rhs=b_rt[:],
                        start=True, stop=False,
                    )
                    for q in range(QB):
                        nc.tensor.matmul(
                            out=o_ps[:, q * D : (q + 1) * D].bitcast(F32R),
                            lhsT=st_sb[:, q * D : (q + 1) * D],
                            rhs=rot_rt[:],
                            is_transpose=True,
                            start=False,
                            stop=(q == QB - 1),
                        )

                    # --- evacuate (plain copy; rotation already applied) ---
                    nc.scalar.copy(out=o_sb[:, sub], in_=o_ps[:])

                # --- store ---
                dst = out[r0 : r0 + DMA_ROWS, :].rearrange("(p q) d -> p q d", p=D)
                nc.sync.dma_start(
                    out=dst, in_=o_sb[:].rearrange("p (q d) -> p q d", d=D)
                )

    nc.compile()
    return nc


def make_consts(w: np.ndarray, W_lin: np.ndarray, b_lin: np.ndarray):
    """Host-side constant preparation (all tiny)."""
    w = np.asarray(w, np.float64)
    c1 = w[:, 0] * w[:, 2]
    c2 = w[:, 1] * w[:, 2]
    # column 1 uses w[1,0] as the outer factor (faithful to source)
    c1[1] = w[1, 0] * w[1, 0]
    c2[1] = w[1, 1] * w[1, 0]

    j = np.arange(D)
    G = np.zeros((D, D), np.float64)
    G[j, (j + 1) % D] += c1
    G[j, (j - 2) % D] -= c2

    rotidx = (j + 1) % D  # row rotation: X~[i] = X[i+1]
    Gt = np.ascontiguousarray(G[rotidx, :].T, np.float32)          # [d, i]
    Wt = np.ascontiguousarray(np.asarray(W_lin, np.float64)[rotidx, :].T, np.float32)
    bvec = np.ascontiguousarray(
        np.tile(np.asarray(b_lin, np.float32), SUB // D)[None, :]
    )
    ones = np.ones((1, D), np.float32)
    ident = np.eye(D, dtype=np.float32)
    # transpose-back permutation: out[b, (i+1)%D] = s~[i, b]
    rotm = np.zeros((D, D), np.float32)
    rotm[j, (j + 1) % D] = 1.0
    return {"gt": Gt, "wt": Wt, "bvec": bvec, "ones": ones, "ident": ident,
            "rot": rotm}


_PROGRAM_CACHE: dict[int, object] = {}
TRACE = False      # test-only: capture NTFF profile on the next kernel() call
TRACE_DIR = None   # test-only: where to keep NTFF/perfetto artifacts
LAST_RESULT = None  # test-only: BassKernelResults of the last run


def _get_program(nrows: int):
    if nrows not in _PROGRAM_CACHE:
        _PROGRAM_CACHE[nrows] = build_program(nrows)
    return _PROGRAM_CACHE[nrows]


def kernel(**inputs) -> np.ndarray:
    inp = np.ascontiguousarray(np.asarray(inputs["inp"], np.float32))
    w = np.asarray(inputs["w"], np.float32)
    W_lin = np.asarray(inputs["W_lin"], np.float32)
    b_lin = np.asarray(inputs["b_lin"], np.float32)

    B = inp.shape[0]
    assert inp.shape[1] == D and B % N_CORES == 0
    nrows = B // N_CORES

    consts = make_consts(w, W_lin, b_lin)
    shards = inp.reshape(N_CORES, nrows, D)

    nc = _get_program(nrows)
    in_maps = [{"inp": shards[i], **consts} for i in range(N_CORES)]
    res = run_bass_kernel_spmd(
        nc, in_maps, list(range(N_CORES)), trace=TRACE, tmpdir=TRACE_DIR
    )
    global LAST_RESULT
    LAST_RESULT = res
    return np.concatenate([res.results[i]["out"] for i in range(N_CORES)], axis=0)


if __name__ == "__main__":
    # quick smoke test on random data vs numpy
    rng = np.random.default_rng(0)
    B = N_CORES * DMA_ROWS * 2
    inp = rng.standard_normal((B, D)).astype(np.float32)
    w = rng.random((D, 3)).astype(np.float32)
    W_lin = (rng.standard_normal((D, D)) / np.sqrt(D)).astype(np.float32)
    b_lin = (rng.standard_normal(D) * 0.01).astype(np.float32)
    dt = np.ones(1, np.float32)

    actual = kernel(inp=inp, dt=dt, w=w, W_lin=W_lin, b_lin=b_lin)

    a = inp.astype(np.float64)
    c1 = (w[:, 0] * w[:, 2]).astype(np.float64)
    c2 = (w[:, 1] * w[:, 2]).astype(np.float64)
    c1[1] = w[1, 0] * w[1, 0]
    c2[1] = w[1, 1] * w[1, 0]
    ap1 = np.roll(a, -1, 1)
    am2 = np.roll(a, 2, 1)
    am1 = np.roll(a, 1, 1)
    x = (c1 * ap1 - c2 * am2) * am1
    expected = x + a @ W_lin.astype(np.float64).T + b_lin
    err = np.abs(actual - expected).max() / np.abs(expected).max()
    print("scale-relative absmax err:", err)


# revision 18
# speedup vs baseline: 1.0182x; 1.0111x over previous
"""Trainium2 Bass kernel for the BINN convnet problem.

Computation (per row b of inp, all column indices mod D=128):
    x[b, j]  = (c1[j] * a[b, j+1] - c2[j] * a[b, j-2]) * a[b, j-1]
    out      = x + a @ W_lin.T + b_lin
with c1[j] = w[j,0]*w[j,2], c2[j] = w[j,1]*w[j,2], except j==1 where the
outer factor is w[1,0] instead of w[1,2].

Strategy: pure data parallel across 8 NeuronCores (batch split).  On each
core, per 512-row compute subtile (1024-row DMA tiles, (p q) layout so each
partition line is one contiguous 4 KB DRAM chunk):

  1. PE-transposes A -> A^T per 128-row block (plain fp32 transpose mode);
     ScalarE evacuates PSUM->SBUF, rounding to float32r.
  2. The stencil's linear part g[b,j] = c1[j] a[b,j+1] - c2[j] a[b,j-2] is
     a constant banded matrix G.  One float32r matmul per block with
     lhsT = A^T-block (stationary) and rhs = [G^T | W_lin^T] (moving,
     N=256 -> full PE rate) produces g and mm = a @ W_lin.T both in
     NATURAL layout in PSUM.  No transpose-back is needed.
  3. DVE: x = a[:, j-1] * g with the j-1 roll expressed as shifted
     free-dim access patterns on the natural A tile (main + 1-col wrap),
     then out = x + mm written straight to SBUF.
  4. GpSimd adds the column bias b_lin (broadcast constant) in SBUF.
  5. Store the natural tile.
"""

import os
import sys

import numpy as np

if os.path.isdir("/opt/trn_rl_repo") and "/opt/trn_rl_repo" not in sys.path:
    sys.path.insert(0, "/opt/trn_rl_repo")

import concourse.mybir as mybir
import concourse.tile as tile
from concourse import bacc
from concourse.bass_utils import run_bass_kernel_spmd

D = 128          # feature dim
N_CORES = 8
SUB = 512        # rows per compute subtile
DMA_ROWS = 1024  # rows per DMA tile
F32 = mybir.dt.float32
F32R = mybir.dt.float32r
BIAS_ON_POOL = True


def build_program(nrows: int):
    """Build the single-core Bass program (SPMD across cores)."""
    assert nrows % DMA_ROWS == 0
    ndma = nrows // DMA_ROWS
    nsub = DMA_ROWS // SUB  # compute subtiles per DMA tile (2)
    QB = SUB // D           # 128-row blocks per compute subtile (4)

    nc = bacc.Bacc("TRN2", debug=False, target_bir_lowering=False)

    inp = nc.declare_dram_parameter("inp", [nrows, D], F32, isOutput=False)
    gw = nc.declare_dram_parameter("gw", [D, 2 * D], F32, isOutput=False)
    bbc = nc.declare_dram_parameter("bbc", [D, DMA_ROWS], F32, isOutput=False)
    bmask = nc.declare_dram_parameter("bmask", [1, SUB], F32, isOutput=False)
    ones = nc.declare_dram_parameter("ones", [1, D], F32, isOutput=False)
    ident = nc.declare_dram_parameter("ident", [D, D], F32, isOutput=False)
    out = nc.declare_dram_parameter("out", [nrows, D], F32, isOutput=True)

    with tile.TileContext(nc) as tc:
        with (
            tc.tile_pool(name="const", bufs=1) as const_pool,
            tc.tile_pool(name="a_sb", bufs=3) as a_pool,
            tc.tile_pool(name="at_sb", bufs=3) as at_pool,
            tc.tile_pool(name="xt_sb", bufs=3) as xt_pool,
            tc.tile_pool(name="o_sb", bufs=3) as o_pool,
            tc.tile_pool(name="at_ps", bufs=3, space="PSUM") as atps_pool,
            tc.tile_pool(name="gw_ps", bufs=2, space="PSUM") as gwps_pool,
        ):
            # --- constants, loaded once ---
            gw_sb = const_pool.tile([D, 2 * D], F32)
            bbc_sb = const_pool.tile([D, DMA_ROWS], F32)
            bmask_sb = const_pool.tile([1, SUB], F32)
            ones_sb = const_pool.tile([1, D], F32)
            id_sb = const_pool.tile([D, D], F32)
            nc.sync.dma_start(out=gw_sb[:], in_=gw[:, :])
            nc.sync.dma_start(out=bbc_sb[:], in_=bbc[:, :])
            nc.sync.dma_start(out=bmask_sb[:], in_=bmask[:, :])
            nc.sync.dma_start(out=ones_sb[:], in_=ones[:, :])
            nc.sync.dma_start(out=id_sb[:], in_=ident[:, :])

            # fp32r matmul operands must be produced by an fp32r-rounding
            # instruction (walrus checkMatmultFP32r) — round the constants once.
            gw_rt = const_pool.tile([D, 2 * D], F32R)
            bmask_rt = const_pool.tile([1, SUB], F32R)
            ones_rt = const_pool.tile([1, D], F32R)
            nc.vector.tensor_copy(out=gw_rt[:], in_=gw_sb[:])
            nc.vector.tensor_copy(out=bmask_rt[:], in_=bmask_sb[:])
            nc.vector.tensor_copy(out=ones_rt[:], in_=ones_sb[:])

            for td in range(ndma):
                r0 = td * DMA_ROWS
                # (p q) layout: partition p holds DMA_ROWS/128 consecutive DRAM
                # rows -> each partition line is one contiguous DRAM chunk.
                a_sb = a_pool.tile([D, DMA_ROWS], F32, tag="a")
                src = inp[r0 : r0 + DMA_ROWS, :].rearrange("(p q) d -> p q d", p=D)
                nc.sync.dma_start(
                    out=a_sb[:].rearrange("p (q d) -> p q d", d=D), in_=src
                )
                o_sb = o_pool.tile([D, DMA_ROWS], F32, tag="o")

                for ts in range(nsub):
                    f0 = ts * SUB

                    # --- PE transpose A -> A^T (per 128 block, plain fp32) ---
                    at_ps = atps_pool.tile([D, SUB], F32, tag="atps")
                    for q in range(QB):
                        nc.tensor.matmul(
                            out=at_ps[:, q * D : (q + 1) * D],
                            lhsT=a_sb[:, f0 + q * D : f0 + (q + 1) * D],
                            rhs=id_sb[:],
                            is_transpose=True,
                            start=True,
                            stop=True,
                        )
                    # evacuate A^T to SBUF (ScalarE), rounding to fp32r
                    at_sb = at_pool.tile([D, SUB], F32R, tag="at")
                    nc.scalar.copy(out=at_sb[:], in_=at_ps[:])

                    # --- [g | mm] per block, natural layout, in PSUM ---
                    # gw_ps free layout: [g0|m0|g1|m1|g2|m2|g3|m3], 2 banks
                    gw_ps = gwps_pool.tile([D, 4 * 2 * D], F32, tag="gwps")
                    for q in range(QB):
                        nc.tensor.matmul(
                            out=gw_ps[:, q * 2 * D : (q + 1) * 2 * D],
                            lhsT=at_sb[:, q * D : (q + 1) * D],
                            rhs=gw_rt[:],
                            start=True,
                            stop=BIAS_ON_POOL,
                        )
                    if not BIAS_ON_POOL:
                        # accumulate b_lin onto the mm halves (masked rhs)
                        for h in range(2):
                            nc.tensor.matmul(
                                out=gw_ps[:, h * SUB : (h + 1) * SUB],
                                lhsT=ones_rt[:],
                                rhs=bmask_rt[:],
                                start=False,
                                stop=True,
                            )

                    gw3 = gw_ps[:].rearrange("p (q c) -> p q c", c=2 * D)
                    a3 = a_sb[:, f0 : f0 + SUB].rearrange("p (q d) -> p q d", d=D)
                    o3 = o_sb[:, f0 : f0 + SUB].rearrange("p (q d) -> p q d", d=D)

                    # --- x = a[:, j-1] * g (DVE; shifted free-dim APs) ---
                    xt_sb = xt_pool.tile([D, SUB], F32, tag="xt")
                    x3 = xt_sb[:].rearrange("p (q d) -> p q d", d=D)
                    nc.vector.tensor_mul(
                        out=x3[:, :, 1:D], in0=a3[:, :, 0 : D - 1],
                        in1=gw3[:, :, 1:D],
                    )
                    nc.vector.tensor_mul(
                        out=x3[:, :, 0:1], in0=a3[:, :, D - 1 : D],
                        in1=gw3[:, :, 0:1],
                    )

                    # --- out = x + mm (DVE, straight to SBUF) ---
                    nc.vector.tensor_add(
                        out=o3[:, :, :], in0=xt_sb[:], in1=gw3[:, :, D : 2 * D]
                    )

                if BIAS_ON_POOL:
                    # --- += b_lin broadcast (GpSimd, SBUF only) ---
                    nc.gpsimd.tensor_tensor(
                        out=o_sb[:], in0=o_sb[:], in1=bbc_sb[:],
                        op=mybir.AluOpType.add,
                    )

                # --- store ---
                dst = out[r0 : r0 + DMA_ROWS, :].rearrange("(p q) d -> p q d", p=D)
                nc.sync.dma_start(
                    out=dst, in_=o_sb[:].rearrange("p (q d) -> p q d", d=D)
                )

    nc.compile()
    return nc


def make_consts(w: np.ndarray, W_lin: np.ndarray, b_lin: np.ndarray):
    """Host-side constant preparation (all tiny)."""
    w = np.asarray(w, np.float64)
    c1 = w[:, 0] * w[:, 2]
    c2 = w[:, 1] * w[:, 2]
    # column 1 uses w[1,0] as the outer factor (faithful to source)
    c1[1] = w[1, 0] * w[1, 0]
    c2[1] = w[1, 1] * w[1, 0]

    j = np.arange(D)
    G = np.zeros((D, D), np.float64)
    G[j, (j + 1) % D] += c1
    G[j, (j - 2) % D] -= c2

    gwm = np.zeros((D, 2 * D), np.float32)
    gwm[:, :D] = G.T           # gw[d, j] = G[j, d]
    gwm[:, D:] = np.asarray(W_lin, np.float64).T  # gw[d, D+j] = W_lin[j, d]

    b32 = np.asarray(b_lin, np.float32)
    bbc = np.ascontiguousarray(np.tile(b32, (D, DMA_ROWS // D)))  # [128, 1024]
    bmask = np.zeros((1, SUB), np.float32)
    bmask[0, D : 2 * D] = b32
    bmask[0, 3 * D : 4 * D] = b32
    ones = np.ones((1, D), np.float32)
    ident = np.eye(D, dtype=np.float32)
    return {"gw": gwm, "bbc": bbc, "bmask": bmask, "ones": ones, "ident": ident}


_PROGRAM_CACHE: dict[int, object] = {}
TRACE = False      # test-only: capture NTFF profile on the next kernel() call
TRACE_DIR = None   # test-only: where to keep NTFF/perfetto artifacts
LAST_RESULT = None  # test-only: BassKernelResults of the last run


def _get_program(nrows: int):
    if nrows not in _PROGRAM_CACHE:
        _PROGRAM_CACHE[nrows] = build_program(nrows)
    return _PROGRAM_CACHE[nrows]


def kernel(**inputs) -> np.ndarray:
    inp = np.ascontiguousarray(np.asarray(inputs["inp"], np.float32))
    w = np.asarray(inputs["w"], np.float32)
    W_lin = np.asarray(inputs["W_lin"], np.float32)
    b_lin = np.asarray(inputs["b_lin"], np.float32)

    B = inp.shape[0]
    assert inp.shape[1] == D and B % N_CORES == 0
    nrows = B // N_CORES

    consts = make_consts(w, W_lin, b_lin)
    shards = inp.reshape(N_CORES, nrows, D)

    nc = _get_program(nrows)
    in_maps = [{"inp": shards[i], **consts} for i in range(N_CORES)]
    res = run_bass_kernel_spmd(
        nc, in_maps, list(range(N_CORES)), trace=TRACE, tmpdir=TRACE_DIR
    )
    global LAST_RESULT
    LAST_RESULT = res
    return np.concatenate([res.results[i]["out"] for i in range(N_CORES)], axis=0)


if __name__ == "__main__":
    # quick smoke test on random data vs numpy
    rng = np.random.default_rng(0)
    B = N_CORES * DMA_ROWS * 2
    inp = rng.standard_normal((B, D)).astype(np.float32)
    w = rng.random((D, 3)).astype(np.float32)
    W_lin = (rng.standard_normal((D, D)) / np.sqrt(D)).astype(np.float32)
    b_lin = (rng.standard_normal(D) * 0.01).astype(np.float32)
    dt = np.ones(1, np.float32)

    actual = kernel(inp=inp, dt=dt, w=w, W_lin=W_lin, b_lin=b_lin)

    a = inp.astype(np.float64)
    c1 = (w[:, 0] * w[:, 2]).astype(np.float64)
    c2 = (w[:, 1] * w[:, 2]).astype(np.float64)
    c1[1] = w[1, 0] * w[1, 0]
    c2[1] = w[1, 1] * w[1, 0]
    ap1 = np.roll(a, -1, 1)
    am2 = np.roll(a, 2, 1)
    am1 = np.roll(a, 1, 1)
    x = (c1 * ap1 - c2 * am2) * am1
    expected = x + a @ W_lin.astype(np.float64).T + b_lin
    err = np.abs(actual - expected).max() / np.abs(expected).max()
    print("scale-relative absmax err:", err)
